# revision 7
# baseline (speedup 1.0000x reference)
"""Causal self-attention (B=2, N=2048, E=1024, H=16, D=64) on 8 TRN2 cores.

Sharding: core c -> batch b = c//4, head group g = c%4 (4 heads = 256
features per core).  Each core computes its heads' q/k/v projections,
causal attention, and a partial out-projection; the host sums the 4
partials per batch.

Per-core dataflow (feature-major "transposed" layouts throughout):
  xT [E, N] (f16)  x  wqkvT slices -> q,k as [feat, tok], v as [tok, feat]
  scoresT [ktok, qtok] = k_h^T-chunks x q_h   (PE row-tiled, 2 heads/pass)
  probsT = exp(scoresT/8) in f16 (ACT), causal triangle masked (DVE)
  attnT [feat, qtok] += v-chunk^T x probsT    (PE col-tiled, 2 heads/pass)
  denom[q] += ones^T x probsT                 (PE col-tiled M=1, 4 heads)
  attnT normalized by PE-broadcast reciprocal; out = attnT^T x woT chunks.
Causal structure skips all fully-masked k-blocks (half the attention
flops); diagonal blocks are computed on their valid q-range only.
"""

import os
import sys
import types

import numpy as np

B, N, E, H, D = 2, 2048, 1024, 16, 64
NCORES = 8


# ---------------------------------------------------------------------------
# Environment patches (this container's walrus accepts only one sync wait per
# instruction; the image's antenv lacks the NTFF profile hook shim).
# ---------------------------------------------------------------------------

def _patch_tile_drain():
    import concourse.mybir as mybir
    import concourse.tile as tile_mod
    from concourse.vector_clock import ScopedClock

    if getattr(tile_mod.TileContext, "_drain_patched", False):
        return

    def _drain_and_barrier(self, tick_clock, wait_clock):
        nc = self.nc
        probe = nc.sync.nop()
        wait_clock.add_sem_waits(probe.ins, ScopedClock({None: tick_clock.global_clock}))
        si = probe.ins.sync_info
        waits = list(si.on_wait) if si and si.on_wait else []
        if len(waits) > 1:
            si.on_wait = waits[:1]
            for w in waits[1:]:
                extra = nc.sync.nop()
                extra.ins.sync_info = mybir.SyncInfo(on_wait=[w], on_update=[])
        nc.sync.drain()
        nc.all_engine_barrier()
        assert self.sems is not None
        popped = nc._tile_sem_poison_stack.pop()
        assert popped is self._sem_poison
        nc.clear_and_free_semaphores(list(self.sems.allocated().values()))
        nc.all_engine_barrier()

    tile_mod.TileContext._drain_and_barrier = _drain_and_barrier
    tile_mod.TileContext._drain_patched = True


def _split_sync_waits(nc, max_waits=1):
    import concourse.mybir as mybir

    cnt = 0
    for f in nc.m.functions:
        for blk in f.blocks:
            insts = blk.instructions
            new = []
            for inst in insts:
                si = inst.sync_info
                waits = list(si.on_wait) if si and si.on_wait else []
                if len(waits) > max_waits:
                    keep = waits[-max_waits:]
                    excess = waits[:-max_waits]
                    for j in range(0, len(excess), max_waits):
                        n = mybir.InstNoOp(name=f"I-ws{cnt}", ins=[], outs=[])
                        cnt += 1
                        n.engine = inst.engine
                        n.sync_info = mybir.SyncInfo(
                            on_wait=excess[j:j + max_waits], on_update=[])
                        new.append(n)
                    si.on_wait = keep
                new.append(inst)
            insts[:] = new
    return cnt


def _install_ntff_shim():
    try:
        import antenv
        if "antenv.axon_hooks" in sys.modules:
            return
        mod = types.ModuleType("antenv.axon_hooks")
        mod._hook = None
        mod.set_axon_ntff_profile_hook = lambda h: setattr(mod, "_hook", h)
        mod.get_axon_ntff_profile_hook = lambda: mod._hook
        sys.modules["antenv.axon_hooks"] = mod
        antenv.axon_hooks = mod
        from trn_agent_boot.trn_boot import _ntff_profile_via_ctypes
        mod._hook = _ntff_profile_via_ctypes("/opt/axon/libaxon_pjrt.so")
    except Exception:
        pass


# ---------------------------------------------------------------------------
# Device program (identical on all 8 cores)
# ---------------------------------------------------------------------------

def _build_nc():
    import concourse.bass as bass
    import concourse.mybir as mybir
    import concourse.tile as tile

    _patch_tile_drain()

    f32 = mybir.dt.float32
    f16 = mybir.dt.float16
    AF = mybir.ActivationFunctionType

    nc = bass.Bass("TRN2", target_bir_lowering=False, debug=False)

    xT = nc.dram_tensor("xT", [E, N], f16, kind="ExternalInput")
    wqkvT = nc.dram_tensor("wqkvT", [E, 768], f16, kind="ExternalInput")
    woT = nc.dram_tensor("woT", [256, E], f16, kind="ExternalInput")
    bqkv = nc.dram_tensor("bqkv", [768, 1], f32, kind="ExternalInput")
    tri = nc.dram_tensor("tri", [128, 2, 128], f16, kind="ExternalInput")
    outp = nc.dram_tensor("outp", [N, E], f16, kind="ExternalOutput")

    NB = N // 512          # 4 token blocks of 512
    NT = N // 128          # 16 token tiles of 128
    NE = E // 128          # 8 contraction chunks
    SCALE = float(D) ** -0.5

    with nc.allow_low_precision(reason="fp16 matmul pipeline"), \
            tile.TileContext(nc) as tc:
        with tc.tile_pool(name="const", bufs=1) as constp, \
                tc.tile_pool(name="qk", bufs=1) as qkp, \
                tc.tile_pool(name="probs", bufs=6) as pbp, \
                tc.tile_pool(name="misc", bufs=2) as miscp, \
                tc.tile_pool(name="stage", bufs=4) as stp, \
                tc.tile_pool(name="mm", bufs=1, space="PSUM") as mmp, \
                tc.tile_pool(name="sc", bufs=2, space="PSUM") as scp, \
                tc.tile_pool(name="pv", bufs=1, space="PSUM") as pvp, \
                tc.tile_pool(name="den", bufs=1, space="PSUM") as denp:

            xT_sb = constp.tile([128, NE, N], f16, tag="xT")
            wq_sb = constp.tile([128, NE, 768], f16, tag="wq")
            wo_sb = constp.tile([128, 2, E], f16, tag="wo")
            bias_sb = constp.tile([128, 6, 1], f32, tag="bias")
            tri_sb = constp.tile([128, 2, 128], f16, tag="tri")
            ones_sb = constp.tile([128, 128], f16, tag="ones")
            q_sb = qkp.tile([128, 2, N], f16, tag="q")
            k_sb = qkp.tile([128, 2, N], f16, tag="k")
            vt_sb = qkp.tile([128, NT, 256], f16, tag="vt")
            at_sb = qkp.tile([128, 2, N], f16, tag="at")

            nc.vector.memset(ones_sb[:], 1.0)
            # ACT table primer: load the exp/ln table set during the DMA ramp
            # (first real exp would otherwise pay ~2.7us mid-pipeline).
            nc.scalar.activation(at_sb[:, 0, 0:8], ones_sb[:, 0:8],
                                 AF.Exp, scale=1.0)
            nc.scalar.activation(at_sb[:, 0, 8:16], ones_sb[:, 8:16], AF.Ln)
            # PE warm-up spinner: junk matmuls keep the PE HAM monitor busy
            # while inputs stream in, so real matmuls start at 2.4 GHz
            # instead of paying ~18us of cold-clock (1.2 GHz) penalty.
            for s in range(10):
                sp = mmp.tile([128, 512], f32, tag="mm", name=f"spin{s}")
                nc.tensor.matmul(sp[0:64, :], ones_sb[:, 0:64],
                                 q_sb[:, 0, 0:512], start=True, stop=True)
            # x on the sync queue, token-block 0 first (first qk unit needs it)
            for nb in range(4):
                for i in range(NE):
                    nc.sync.dma_start(
                        xT_sb[:, i, nb * 512:(nb + 1) * 512],
                        xT.ap()[i * 128:(i + 1) * 128, nb * 512:(nb + 1) * 512])
            # qkv weights on the gpsimd queue (runs concurrently with x);
            # scalar issues no DMA at all -- it is the exp bottleneck engine.
            for i in range(NE):
                nc.gpsimd.dma_start(wq_sb[:, i, :], wqkvT.ap()[i * 128:(i + 1) * 128, :])
            # small loads on scalar: ~3us of DMA, done before its first exp
            for i in range(6):
                nc.scalar.dma_start(bias_sb[:, i, :], bqkv.ap()[i * 128:(i + 1) * 128, :])
            nc.scalar.dma_start(tri_sb[:], tri.ap())
            for i in range(2):
                nc.scalar.dma_start(wo_sb[:, i, :], woT.ap()[i * 128:(i + 1) * 128, :])

            # -- filler work units (emitted interleaved into attention) ----
            def emit_qk_unit(ft, nb):
                # q/k feature tile ft (0,1=q; 2,3=k), token block nb
                ps = mmp.tile([128, 512], f32, tag="mm", name=f"qk_{ft}_{nb}")
                for e in range(NE):
                    nc.tensor.matmul(
                        ps[:], wq_sb[:, e, ft * 128:(ft + 1) * 128],
                        xT_sb[:, e, nb * 512:(nb + 1) * 512],
                        start=(e == 0), stop=(e == NE - 1))
                dest = (q_sb if ft < 2 else k_sb)[:, ft % 2,
                                                  nb * 512:(nb + 1) * 512]
                nc.vector.tensor_scalar_add(dest, ps[:], bias_sb[:, ft, :])

            def emit_v_unit(tt):
                ps = mmp.tile([128, 256], f32, tag="mm", name=f"v_{tt}")
                for e in range(NE):
                    nc.tensor.matmul(
                        ps[:], xT_sb[:, e, tt * 128:(tt + 1) * 128],
                        wq_sb[:, e, 512:768],
                        start=(e == 0), stop=(e == NE - 1))
                nc.vector.tensor_copy(vt_sb[:, tt, :], ps[:])

            store_ctr = [0]

            def emit_store(ap_out, st):
                # round-robin output stores over sync/sync/gpsimd queues
                eng = nc.gpsimd if store_ctr[0] % 3 == 2 else nc.sync
                store_ctr[0] += 1
                eng.dma_start(ap_out, st)

            def emit_p3_unit(tt):
                for n2 in range(2):
                    ps = mmp.tile([128, 512], f32, tag="mm",
                                  name=f"p3_{tt}_{n2}")
                    for fp in range(2):
                        nc.tensor.matmul(
                            ps[:],
                            at_sb[:, fp, tt * 128:(tt + 1) * 128],
                            wo_sb[:, fp, n2 * 512:(n2 + 1) * 512],
                            start=(fp == 0), stop=(fp == 1))
                    st = stp.tile([128, 512], f16, tag="st",
                                  name=f"st_{tt}_{n2}")
                    nc.vector.tensor_copy(st[:], ps[:])
                    emit_store(
                        outp.ap()[tt * 128:(tt + 1) * 128,
                                  n2 * 512:(n2 + 1) * 512], st[:])

            def emit_norm_b(item):
                # PE broadcast of 1/denom + normalize into attnT (+v bias)
                jj, araw, rec = item
                js = slice(512 * jj, 512 * (jj + 1))
                for p in (0, 1):
                    bc = mmp.tile([128, 512], f32, tag="mm",
                                  name=f"bc_{jj}_{p}")
                    for hh in (0, 1):
                        h = 2 * p + hh
                        nc.tensor.matmul(
                            bc[64 * hh:64 * hh + 64, :],
                            ones_sb[32 * h:32 * h + 1, 0:64],
                            rec[32 * h:32 * h + 1, :],
                            start=True, stop=True,
                            tile_position=(32 * h, 64 * hh))
                    nc.vector.tensor_mul(at_sb[:, p, js], araw[p][:], bc[:])
                    nc.vector.tensor_scalar_add(at_sb[:, p, js],
                                                at_sb[:, p, js],
                                                bias_sb[:, 4 + p, :])

            # -- p1 block 0 up front (attention j=0 needs it) --------------
            for ft in range(4):
                emit_qk_unit(ft, 0)
            for tt in range(4):
                emit_v_unit(tt)

            # -- attention blocks: depth-2 software pipeline ---------------
            # scores/exp for k-block ik+2 are emitted while pv/den of block
            # ik execute, so ACT (exp) and PE overlap instead of ping-pong.
            pending = None
            for j in range(NB):
                if j == 0:
                    fillers = [(emit_qk_unit, (ft, 1)) for ft in range(4)]
                    fillers += [(emit_v_unit, (tt,)) for tt in range(4, 8)]
                elif j == 1:
                    fillers = [(emit_qk_unit, (ft, 2)) for ft in range(4)]
                    fillers += [(emit_v_unit, (tt,)) for tt in range(8, 12)]
                elif j == 2:
                    fillers = [(emit_qk_unit, (ft, 3)) for ft in range(4)]
                    fillers += [(emit_v_unit, (tt,)) for tt in range(12, 16)]
                    fillers += [(emit_p3_unit, (tt,)) for tt in range(4)]
                else:
                    fillers = [(emit_p3_unit, (tt,)) for tt in range(4, 12)]
                if pending is not None:
                    fillers.insert(min(2, len(fillers)), (emit_norm_b, (pending,)))
                    pending = None
                nf = len(fillers)
                pv_ps = [pvp.tile([128, 512], f32, tag=f"pv{p}",
                                  name=f"pv{p}_{j}") for p in (0, 1)]
                den_ps = denp.tile([128, 512], f32, tag="den",
                                   name=f"den_{j}")
                nk = 4 * (j + 1)

                def emit_scores(ik, j=j):
                    r = ik - 4 * j
                    qoff = 128 * r if r > 0 else 0
                    qs = slice(512 * j + qoff, 512 * (j + 1))
                    pbs = []
                    for p in (0, 1):
                        sc = scp.tile([128, 2, 512], f32, tag="sc",
                                      name=f"sc_{j}_{ik}_{p}")
                        for hh in (0, 1):
                            dsl = slice(64 * hh, 64 * hh + 64)
                            nc.tensor.matmul(
                                sc[:, hh, qoff:512],
                                k_sb[dsl, p, ik * 128:(ik + 1) * 128],
                                q_sb[dsl, p, qs],
                                start=True, stop=True)
                        pb = pbp.tile([128, 2, 512], f16, tag="pb",
                                      name=f"pb_{j}_{ik}_{p}")
                        nc.scalar.activation(pb[:, :, qoff:512],
                                             sc[:, :, qoff:512],
                                             AF.Exp, scale=SCALE)
                        if r >= 0:
                            nc.gpsimd.tensor_mul(
                                pb[:, :, qoff:qoff + 128],
                                pb[:, :, qoff:qoff + 128], tri_sb[:])
                        pbs.append(pb)
                    return pbs

                stage = {0: emit_scores(0)}
                if nk > 1:
                    stage[1] = emit_scores(1)
                fdone = 0
                for ik in range(nk):
                    r = ik - 4 * j
                    qoff = 128 * r if r > 0 else 0
                    first, last = ik == 0, ik == nk - 1
                    pbs = stage.pop(ik)
                    for p in (0, 1):
                        for hh in (0, 1):
                            h = 2 * p + hh
                            nc.tensor.matmul(
                                pv_ps[p][64 * hh:64 * hh + 64, qoff:512],
                                vt_sb[:, ik, 64 * h:64 * h + 64],
                                pbs[p][:, hh, qoff:512],
                                start=first, stop=last,
                                tile_position=(0, 64 * hh),
                                skip_group_check=True)
                    for h in range(4):
                        nc.tensor.matmul(
                            den_ps[32 * h:32 * h + 1, qoff:512],
                            ones_sb[:, 0:1],
                            pbs[h // 2][:, h % 2, qoff:512],
                            start=first, stop=last,
                            tile_position=(0, 32 * h),
                            skip_group_check=True)
                    want = ((ik + 1) * nf) // nk
                    while fdone < want:
                        fn, args = fillers[fdone]
                        fn(*args)
                        fdone += 1
                    if ik + 2 < nk:
                        stage[ik + 2] = emit_scores(ik + 2)
                araw = []
                for p in (0, 1):
                    ar = miscp.tile([128, 512], f32, tag=f"araw{p}",
                                    name=f"araw{p}_{j}")
                    nc.vector.tensor_copy(ar[:], pv_ps[p][:])
                    araw.append(ar)
                den_sb = miscp.tile([128, 512], f32, tag="densb",
                                    name=f"densb_{j}")
                nc.vector.tensor_copy(den_sb[0:97, :], den_ps[0:97, :])
                rec = miscp.tile([128, 512], f16, tag="rec", name=f"rec_{j}")
                if j == NB - 1:
                    # tail-critical: 1/x = exp(-ln x) on ACT (1.4us) instead
                    # of the DVE iterative reciprocal (3.4us)
                    lnden = miscp.tile([128, 512], f32, tag="lnden",
                                       name=f"lnden_{j}")
                    nc.scalar.activation(lnden[0:97, :], den_sb[0:97, :],
                                         AF.Ln)
                    nc.scalar.activation(rec[0:97, :], lnden[0:97, :],
                                         AF.Exp, scale=-1.0)
                else:
                    nc.vector.reciprocal(rec[0:97, :], den_sb[0:97, :])
                pending = (j, araw, rec)

            # -- tail: final normalize + last output tiles -----------------
            emit_norm_b(pending)
            if True:
                for tt in range(12, NT):
                    for n2 in range(2):
                        ps = scp.tile([128, 512], f32, tag="sc",
                                      name=f"p3t_{tt}_{n2}")
                        for fp in range(2):
                            nc.tensor.matmul(
                                ps[:],
                                at_sb[:, fp, tt * 128:(tt + 1) * 128],
                                wo_sb[:, fp, n2 * 512:(n2 + 1) * 512],
                                start=(fp == 0), stop=(fp == 1))
                        st = stp.tile([128, 512], f16, tag="st",
                                      name=f"stt_{tt}_{n2}")
                        nc.vector.tensor_copy(st[:], ps[:])
                        emit_store(
                            outp.ap()[tt * 128:(tt + 1) * 128,
                                      n2 * 512:(n2 + 1) * 512], st[:])

    _split_sync_waits(nc)
    return nc


_NC = None


def _get_nc():
    global _NC
    if _NC is None:
        _NC = _build_nc()
    return _NC


# ---------------------------------------------------------------------------
# Host entry point
# ---------------------------------------------------------------------------

def kernel(x, qkv_w, qkv_b, out_w, out_b):
    from concourse.bass_utils import run_bass_kernel_spmd

    trace_dir = os.environ.get("BASS_KERNEL_TRACE_DIR")
    if trace_dir:
        _install_ntff_shim()

    nc = _get_nc()

    x = np.asarray(x, np.float32)
    qkv_w = np.asarray(qkv_w, np.float32)
    qkv_b = np.asarray(qkv_b, np.float32)
    out_w = np.asarray(out_w, np.float32)
    out_b = np.asarray(out_b, np.float32)

    tri_np = np.broadcast_to(np.triu(np.ones((128, 128), np.float16))[:, None, :],
        (128, 2, 128)).copy()
    in_maps = []
    for c in range(NCORES):
        b, g = divmod(c, 4)
        fs = slice(256 * g, 256 * g + 256)
        wqkvT = np.ascontiguousarray(
            np.concatenate([qkv_w[0 * E:1 * E][fs],
                            qkv_w[1 * E:2 * E][fs],
                            qkv_w[2 * E:3 * E][fs]], axis=0).T)
        bq = np.concatenate([qkv_b[0 * E:1 * E][fs],
                             qkv_b[1 * E:2 * E][fs],
                             qkv_b[2 * E:3 * E][fs]])[:, None]
        in_maps.append({
            "xT": np.ascontiguousarray(x[b].T).astype(np.float16),
            "wqkvT": wqkvT.astype(np.float16),
            "woT": np.ascontiguousarray(out_w[:, fs].T).astype(np.float16),
            "bqkv": np.ascontiguousarray(bq),
            "tri": tri_np,
        })

    kwargs = {}
    if trace_dir:
        kwargs = {"trace": True, "tmpdir": trace_dir}
    res = run_bass_kernel_spmd(nc, in_maps, core_ids=list(range(NCORES)), **kwargs)
    if trace_dir and res.exec_time_ns is not None:
        print(f"HW exec time: {res.exec_time_ns} ns")

    out = np.zeros((B, N, E), np.float32)
    for c in range(NCORES):
        out[c // 4] += res.results[c]["outp"].astype(np.float32)
    out += out_b[None, None, :]
    return out



# revision 12
# speedup vs baseline: 1.0286x; 1.0286x over previous
"""Causal self-attention (B=2, N=2048, E=1024, H=16, D=64) on 8 TRN2 cores.

Sharding: core c -> batch b = c//4, head group g = c%4 (4 heads = 256
features per core).  Each core computes its heads' q/k/v projections,
causal attention, and a partial out-projection; the host sums the 4
partials per batch.

Per-core dataflow (feature-major "transposed" layouts throughout):
  xT [E, N] (f16)  x  wqkvT slices -> q,k as [feat, tok], v as [tok, feat]
  scoresT [ktok, qtok] = k_h^T-chunks x q_h   (PE row-tiled, 2 heads/pass)
  probsT = exp(scoresT/8) in f16 (ACT), causal triangle masked (DVE)
  attnT [feat, qtok] += v-chunk^T x probsT    (PE col-tiled, 2 heads/pass)
  denom[q] += ones^T x probsT                 (PE col-tiled M=1, 4 heads)
  attnT normalized by PE-broadcast reciprocal; out = attnT^T x woT chunks.
Causal structure skips all fully-masked k-blocks (half the attention
flops); diagonal blocks are computed on their valid q-range only.
"""

import os
import sys
import types

import numpy as np

B, N, E, H, D = 2, 2048, 1024, 16, 64
NCORES = 8


# ---------------------------------------------------------------------------
# Environment patches (this container's walrus accepts only one sync wait per
# instruction; the image's antenv lacks the NTFF profile hook shim).
# ---------------------------------------------------------------------------

def _patch_tile_drain():
    import concourse.mybir as mybir
    import concourse.tile as tile_mod
    from concourse.vector_clock import ScopedClock

    if getattr(tile_mod.TileContext, "_drain_patched", False):
        return

    def _drain_and_barrier(self, tick_clock, wait_clock):
        nc = self.nc
        probe = nc.sync.nop()
        wait_clock.add_sem_waits(probe.ins, ScopedClock({None: tick_clock.global_clock}))
        si = probe.ins.sync_info
        waits = list(si.on_wait) if si and si.on_wait else []
        if len(waits) > 1:
            si.on_wait = waits[:1]
            for w in waits[1:]:
                extra = nc.sync.nop()
                extra.ins.sync_info = mybir.SyncInfo(on_wait=[w], on_update=[])
        nc.sync.drain()
        nc.all_engine_barrier()
        assert self.sems is not None
        popped = nc._tile_sem_poison_stack.pop()
        assert popped is self._sem_poison
        nc.clear_and_free_semaphores(list(self.sems.allocated().values()))
        nc.all_engine_barrier()

    tile_mod.TileContext._drain_and_barrier = _drain_and_barrier
    tile_mod.TileContext._drain_patched = True


def _split_sync_waits(nc, max_waits=1):
    import concourse.mybir as mybir

    cnt = 0
    for f in nc.m.functions:
        for blk in f.blocks:
            insts = blk.instructions
            new = []
            for inst in insts:
                si = inst.sync_info
                waits = list(si.on_wait) if si and si.on_wait else []
                if len(waits) > max_waits:
                    keep = waits[-max_waits:]
                    excess = waits[:-max_waits]
                    for j in range(0, len(excess), max_waits):
                        n = mybir.InstNoOp(name=f"I-ws{cnt}", ins=[], outs=[])
                        cnt += 1
                        n.engine = inst.engine
                        n.sync_info = mybir.SyncInfo(
                            on_wait=excess[j:j + max_waits], on_update=[])
                        new.append(n)
                    si.on_wait = keep
                new.append(inst)
            insts[:] = new
    return cnt


def _install_ntff_shim():
    try:
        import antenv
        if "antenv.axon_hooks" in sys.modules:
            return
        mod = types.ModuleType("antenv.axon_hooks")
        mod._hook = None
        mod.set_axon_ntff_profile_hook = lambda h: setattr(mod, "_hook", h)
        mod.get_axon_ntff_profile_hook = lambda: mod._hook
        sys.modules["antenv.axon_hooks"] = mod
        antenv.axon_hooks = mod
        from trn_agent_boot.trn_boot import _ntff_profile_via_ctypes
        mod._hook = _ntff_profile_via_ctypes("/opt/axon/libaxon_pjrt.so")
    except Exception:
        pass


# ---------------------------------------------------------------------------
# Device program (identical on all 8 cores)
# ---------------------------------------------------------------------------

def _build_nc():
    import concourse.bass as bass
    import concourse.mybir as mybir
    import concourse.tile as tile

    _patch_tile_drain()

    f32 = mybir.dt.float32
    f16 = mybir.dt.float16
    AF = mybir.ActivationFunctionType

    nc = bass.Bass("TRN2", target_bir_lowering=False, debug=False)

    xT = nc.dram_tensor("xT", [E, N], f16, kind="ExternalInput")
    wqkvT = nc.dram_tensor("wqkvT", [E, 768], f16, kind="ExternalInput")
    woT = nc.dram_tensor("woT", [256, E], f16, kind="ExternalInput")
    bqkv = nc.dram_tensor("bqkv", [768, 1], f32, kind="ExternalInput")
    tri = nc.dram_tensor("tri", [128, 2, 128], f16, kind="ExternalInput")
    outp = nc.dram_tensor("outp", [N, E], f16, kind="ExternalOutput")

    NB = N // 512          # 4 token blocks of 512
    NT = N // 128          # 16 token tiles of 128
    NE = E // 128          # 8 contraction chunks
    SCALE = float(D) ** -0.5

    with nc.allow_low_precision(reason="fp16 matmul pipeline"), \
            tile.TileContext(nc) as tc:
        with tc.tile_pool(name="const", bufs=1) as constp, \
                tc.tile_pool(name="qk", bufs=1) as qkp, \
                tc.tile_pool(name="probs", bufs=6) as pbp, \
                tc.tile_pool(name="misc", bufs=2) as miscp, \
                tc.tile_pool(name="stage", bufs=4) as stp, \
                tc.tile_pool(name="mm", bufs=1, space="PSUM") as mmp, \
                tc.tile_pool(name="sc", bufs=2, space="PSUM") as scp, \
                tc.tile_pool(name="pv", bufs=1, space="PSUM") as pvp, \
                tc.tile_pool(name="den", bufs=1, space="PSUM") as denp:

            xT_sb = constp.tile([128, NE, N], f16, tag="xT")
            wq_sb = constp.tile([128, NE, 768], f16, tag="wq")
            wo_sb = constp.tile([128, 2, E], f16, tag="wo")
            bias_sb = constp.tile([128, 6, 1], f32, tag="bias")
            tri_sb = constp.tile([128, 2, 128], f16, tag="tri")
            ones_sb = constp.tile([128, 128], f16, tag="ones")
            q_sb = qkp.tile([128, 2, N], f16, tag="q")
            k_sb = qkp.tile([128, 2, N], f16, tag="k")
            vt_sb = qkp.tile([128, NT, 256], f16, tag="vt")
            at_sb = qkp.tile([128, 2, N], f16, tag="at")

            nc.vector.memset(ones_sb[:], 1.0)
            # ACT table primer: load the exp/ln table set during the DMA ramp
            # (first real exp would otherwise pay ~2.7us mid-pipeline).
            nc.scalar.activation(at_sb[:, 0, 0:8], ones_sb[:, 0:8],
                                 AF.Exp, scale=1.0)
            nc.scalar.activation(at_sb[:, 0, 8:16], ones_sb[:, 8:16], AF.Ln)
            # PE warm-up spinner: gapless full-array accumulation chains keep
            # the PE HAM monitor busy while inputs stream in, so real matmuls
            # start at 2.4 GHz instead of paying the cold-clock (1.2 GHz)
            # penalty.  The HAM only un-throttles after ~3.4us of PE activity
            # with no gaps, so mimic the qkv unit structure (8-matmul
            # accumulation chains).
            for s in range(6):
                sp = mmp.tile([128, 512], f32, tag="mm", name=f"spin{s}")
                for e in range(8):
                    nc.tensor.matmul(sp[:, 0:128], ones_sb[:],
                                     q_sb[:, 0, 0:128],
                                     start=(e == 0), stop=(e == 7))
            # x on the sync queue, token-block 0 first (first qk unit needs it)
            for nb in range(4):
                for i in range(NE):
                    nc.sync.dma_start(
                        xT_sb[:, i, nb * 512:(nb + 1) * 512],
                        xT.ap()[i * 128:(i + 1) * 128, nb * 512:(nb + 1) * 512])
            # qkv weights on the gpsimd queue (runs concurrently with x);
            # scalar issues no DMA at all -- it is the exp bottleneck engine.
            for i in range(NE):
                nc.gpsimd.dma_start(wq_sb[:, i, :], wqkvT.ap()[i * 128:(i + 1) * 128, :])
            # small loads on scalar: ~3us of DMA, done before its first exp
            for i in range(6):
                nc.scalar.dma_start(bias_sb[:, i, :], bqkv.ap()[i * 128:(i + 1) * 128, :])
            nc.scalar.dma_start(tri_sb[:], tri.ap())
            for i in range(2):
                nc.scalar.dma_start(wo_sb[:, i, :], woT.ap()[i * 128:(i + 1) * 128, :])

            # -- filler work units (emitted interleaved into attention) ----
            def emit_qk_unit(ft, nb):
                # q/k feature tile ft (0,1=q; 2,3=k), token block nb
                ps = mmp.tile([128, 512], f32, tag="mm", name=f"qk_{ft}_{nb}")
                for e in range(NE):
                    nc.tensor.matmul(
                        ps[:], wq_sb[:, e, ft * 128:(ft + 1) * 128],
                        xT_sb[:, e, nb * 512:(nb + 1) * 512],
                        start=(e == 0), stop=(e == NE - 1))
                dest = (q_sb if ft < 2 else k_sb)[:, ft % 2,
                                                  nb * 512:(nb + 1) * 512]
                nc.vector.tensor_scalar_add(dest, ps[:], bias_sb[:, ft, :])

            def emit_v_unit(tt):
                ps = mmp.tile([128, 256], f32, tag="mm", name=f"v_{tt}")
                for e in range(NE):
                    nc.tensor.matmul(
                        ps[:], xT_sb[:, e, tt * 128:(tt + 1) * 128],
                        wq_sb[:, e, 512:768],
                        start=(e == 0), stop=(e == NE - 1))
                nc.vector.tensor_copy(vt_sb[:, tt, :], ps[:])

            store_ctr = [0]

            def emit_store(ap_out, st):
                # round-robin output stores over sync/sync/gpsimd queues
                eng = nc.gpsimd if store_ctr[0] % 3 == 2 else nc.sync
                store_ctr[0] += 1
                eng.dma_start(ap_out, st)

            def emit_p3_unit(tt):
                for n2 in range(2):
                    ps = mmp.tile([128, 512], f32, tag="mm",
                                  name=f"p3_{tt}_{n2}")
                    for fp in range(2):
                        nc.tensor.matmul(
                            ps[:],
                            at_sb[:, fp, tt * 128:(tt + 1) * 128],
                            wo_sb[:, fp, n2 * 512:(n2 + 1) * 512],
                            start=(fp == 0), stop=(fp == 1))
                    st = stp.tile([128, 512], f16, tag="st",
                                  name=f"st_{tt}_{n2}")
                    nc.vector.tensor_copy(st[:], ps[:])
                    emit_store(
                        outp.ap()[tt * 128:(tt + 1) * 128,
                                  n2 * 512:(n2 + 1) * 512], st[:])

            def emit_norm_b(item, tail=False):
                # PE broadcast of 1/denom + normalize into attnT (+v bias)
                jj, araw, rec = item
                js = slice(512 * jj, 512 * (jj + 1))
                for p in (0, 1):
                    if tail:
                        # scores pool is free at the tail: its 2 buffers let
                        # bc(p1) run without serializing behind mul(p0) on
                        # the single-buffer mm pool
                        bct = scp.tile([128, 2, 512], f32, tag="sc",
                                       name=f"bc_{jj}_{p}")
                        bc_hh = lambda hh: bct[64 * hh:64 * hh + 64, 0, :]
                        bc_all = bct[:, 0, :]
                    else:
                        bcm = mmp.tile([128, 512], f32, tag="mm",
                                       name=f"bc_{jj}_{p}")
                        bc_hh = lambda hh: bcm[64 * hh:64 * hh + 64, :]
                        bc_all = bcm[:]
                    for hh in (0, 1):
                        h = 2 * p + hh
                        nc.tensor.matmul(
                            bc_hh(hh),
                            ones_sb[32 * h:32 * h + 1, 0:64],
                            rec[32 * h:32 * h + 1, :],
                            start=True, stop=True,
                            tile_position=(32 * h, 64 * hh))
                    nc.vector.tensor_mul(at_sb[:, p, js], araw[p][:], bc_all)
                    nc.vector.tensor_scalar_add(at_sb[:, p, js],
                                                at_sb[:, p, js],
                                                bias_sb[:, 4 + p, :])

            # -- p1 block 0 up front (attention j=0 needs it) --------------
            for ft in range(4):
                emit_qk_unit(ft, 0)
            for tt in range(4):
                emit_v_unit(tt)

            # -- attention blocks: depth-2 software pipeline ---------------
            # scores/exp for k-block ik+2 are emitted while pv/den of block
            # ik execute, so ACT (exp) and PE overlap instead of ping-pong.
            pending = None
            for j in range(NB):
                if j == 0:
                    fillers = [(emit_qk_unit, (ft, 1)) for ft in range(4)]
                    fillers += [(emit_v_unit, (tt,)) for tt in range(4, 8)]
                elif j == 1:
                    fillers = [(emit_qk_unit, (ft, 2)) for ft in range(4)]
                    fillers += [(emit_v_unit, (tt,)) for tt in range(8, 12)]
                elif j == 2:
                    fillers = [(emit_qk_unit, (ft, 3)) for ft in range(4)]
                    fillers += [(emit_v_unit, (tt,)) for tt in range(12, 16)]
                    fillers += [(emit_p3_unit, (tt,)) for tt in range(4)]
                else:
                    fillers = [(emit_p3_unit, (tt,)) for tt in range(4, 12)]
                if pending is not None:
                    fillers.insert(min(2, len(fillers)), (emit_norm_b, (pending,)))
                    pending = None
                nf = len(fillers)
                pv_ps = [pvp.tile([128, 512], f32, tag=f"pv{p}",
                                  name=f"pv{p}_{j}") for p in (0, 1)]
                den_ps = denp.tile([128, 512], f32, tag="den",
                                   name=f"den_{j}")
                nk = 4 * (j + 1)

                def emit_scores(ik, j=j):
                    r = ik - 4 * j
                    qoff = 128 * r if r > 0 else 0
                    qs = slice(512 * j + qoff, 512 * (j + 1))
                    pbs = []
                    for p in (0, 1):
                        sc = scp.tile([128, 2, 512], f32, tag="sc",
                                      name=f"sc_{j}_{ik}_{p}")
                        for hh in (0, 1):
                            dsl = slice(64 * hh, 64 * hh + 64)
                            nc.tensor.matmul(
                                sc[:, hh, qoff:512],
                                k_sb[dsl, p, ik * 128:(ik + 1) * 128],
                                q_sb[dsl, p, qs],
                                start=True, stop=True)
                        pb = pbp.tile([128, 2, 512], f16, tag="pb",
                                      name=f"pb_{j}_{ik}_{p}")
                        nc.scalar.activation(pb[:, :, qoff:512],
                                             sc[:, :, qoff:512],
                                             AF.Exp, scale=SCALE)
                        if r >= 0:
                            nc.gpsimd.tensor_mul(
                                pb[:, :, qoff:qoff + 128],
                                pb[:, :, qoff:qoff + 128], tri_sb[:])
                        pbs.append(pb)
                    return pbs

                stage = {0: emit_scores(0)}
                if nk > 1:
                    stage[1] = emit_scores(1)
                fdone = 0
                for ik in range(nk):
                    r = ik - 4 * j
                    qoff = 128 * r if r > 0 else 0
                    first, last = ik == 0, ik == nk - 1
                    pbs = stage.pop(ik)
                    for p in (0, 1):
                        for hh in (0, 1):
                            h = 2 * p + hh
                            nc.tensor.matmul(
                                pv_ps[p][64 * hh:64 * hh + 64, qoff:512],
                                vt_sb[:, ik, 64 * h:64 * h + 64],
                                pbs[p][:, hh, qoff:512],
                                start=first, stop=last,
                                tile_position=(0, 64 * hh),
                                skip_group_check=True)
                    for h in range(4):
                        nc.tensor.matmul(
                            den_ps[32 * h:32 * h + 1, qoff:512],
                            ones_sb[:, 0:1],
                            pbs[h // 2][:, h % 2, qoff:512],
                            start=first, stop=last,
                            tile_position=(0, 32 * h),
                            skip_group_check=True)
                    want = ((ik + 1) * nf) // nk
                    while fdone < want:
                        fn, args = fillers[fdone]
                        fn(*args)
                        fdone += 1
                    if ik + 2 < nk:
                        stage[ik + 2] = emit_scores(ik + 2)
                araw = []
                for p in (0, 1):
                    ar = miscp.tile([128, 512], f32, tag=f"araw{p}",
                                    name=f"araw{p}_{j}")
                    nc.vector.tensor_copy(ar[:], pv_ps[p][:])
                    araw.append(ar)
                den_sb = miscp.tile([128, 512], f32, tag="densb",
                                    name=f"densb_{j}")
                nc.vector.tensor_copy(den_sb[0:97, :], den_ps[0:97, :])
                rec = miscp.tile([128, 512], f16, tag="rec", name=f"rec_{j}")
                if j == NB - 1:
                    # tail-critical: 1/x = exp(-ln x) on ACT (1.4us) instead
                    # of the DVE iterative reciprocal (3.4us)
                    lnden = miscp.tile([128, 512], f32, tag="lnden",
                                       name=f"lnden_{j}")
                    nc.scalar.activation(lnden[0:97, :], den_sb[0:97, :],
                                         AF.Ln)
                    nc.scalar.activation(rec[0:97, :], lnden[0:97, :],
                                         AF.Exp, scale=-1.0)
                else:
                    nc.vector.reciprocal(rec[0:97, :], den_sb[0:97, :])
                pending = (j, araw, rec)

            # -- tail: final normalize + last output tiles -----------------
            emit_norm_b(pending, tail=True)
            for idx, tt in enumerate(range(12, NT)):
                for n2 in range(2):
                    # rotate psum across the (now free) sc + mm + pv pools
                    # so matmuls are not gated on the previous tile's cast
                    k = (2 * idx + n2) % 4
                    if k < 2:
                        pst = scp.tile([128, 2, 512], f32, tag="sc",
                                       name=f"p3t_{tt}_{n2}")
                        ps = pst[:, 0, :]
                    elif k == 2:
                        ps = mmp.tile([128, 512], f32, tag="mm",
                                      name=f"p3t_{tt}_{n2}")[:]
                    else:
                        ps = pvp.tile([128, 512], f32, tag="pv0",
                                      name=f"p3t_{tt}_{n2}")[:]
                    for fp in range(2):
                        nc.tensor.matmul(
                            ps,
                            at_sb[:, fp, tt * 128:(tt + 1) * 128],
                            wo_sb[:, fp, n2 * 512:(n2 + 1) * 512],
                            start=(fp == 0), stop=(fp == 1))
                    st = stp.tile([128, 512], f16, tag="st",
                                  name=f"stt_{tt}_{n2}")
                    # split tail psum evacuation across both cast engines
                    # (scalar is exp-free by now)
                    ceng = nc.scalar if (2 * idx + n2) % 2 else nc.vector
                    if ceng is nc.scalar:
                        nc.scalar.activation(st[:], ps, AF.Copy)
                    else:
                        nc.vector.tensor_copy(st[:], ps)
                    emit_store(
                        outp.ap()[tt * 128:(tt + 1) * 128,
                                  n2 * 512:(n2 + 1) * 512], st[:])

    _split_sync_waits(nc)
    return nc


_NC = None


def _get_nc():
    global _NC
    if _NC is None:
        _NC = _build_nc()
    return _NC


# ---------------------------------------------------------------------------
# Host entry point
# ---------------------------------------------------------------------------

def kernel(x, qkv_w, qkv_b, out_w, out_b):
    from concourse.bass_utils import run_bass_kernel_spmd

    trace_dir = os.environ.get("BASS_KERNEL_TRACE_DIR")
    if trace_dir:
        _install_ntff_shim()

    nc = _get_nc()

    x = np.asarray(x, np.float32)
    qkv_w = np.asarray(qkv_w, np.float32)
    qkv_b = np.asarray(qkv_b, np.float32)
    out_w = np.asarray(out_w, np.float32)
    out_b = np.asarray(out_b, np.float32)

    tri_np = np.broadcast_to(np.triu(np.ones((128, 128), np.float16))[:, None, :],
        (128, 2, 128)).copy()
    in_maps = []
    for c in range(NCORES):
        b, g = divmod(c, 4)
        fs = slice(256 * g, 256 * g + 256)
        wqkvT = np.ascontiguousarray(
            np.concatenate([qkv_w[0 * E:1 * E][fs],
                            qkv_w[1 * E:2 * E][fs],
                            qkv_w[2 * E:3 * E][fs]], axis=0).T)
        bq = np.concatenate([qkv_b[0 * E:1 * E][fs],
                             qkv_b[1 * E:2 * E][fs],
                             qkv_b[2 * E:3 * E][fs]])[:, None]
        in_maps.append({
            "xT": np.ascontiguousarray(x[b].T).astype(np.float16),
            "wqkvT": wqkvT.astype(np.float16),
            "woT": np.ascontiguousarray(out_w[:, fs].T).astype(np.float16),
            "bqkv": np.ascontiguousarray(bq),
            "tri": tri_np,
        })

    kwargs = {}
    if trace_dir:
        kwargs = {"trace": True, "tmpdir": trace_dir}
    res = run_bass_kernel_spmd(nc, in_maps, core_ids=list(range(NCORES)), **kwargs)
    if trace_dir and res.exec_time_ns is not None:
        print(f"HW exec time: {res.exec_time_ns} ns")

    out = np.zeros((B, N, E), np.float32)
    for c in range(NCORES):
        out[c // 4] += res.results[c]["outp"].astype(np.float32)
    out += out_b[None, None, :]
    return out



# revision 17
# speedup vs baseline: 1.1209x; 1.0897x over previous
"""Causal self-attention (B=2, N=2048, E=1024, H=16, D=64) on 8 TRN2 cores.

Sharding: core c -> batch b = c//4, head group g = c%4 (4 heads = 256
features per core).  Each core computes its heads' q/k/v projections,
causal attention, and a partial out-projection; the host sums the 4
partials per batch.

Per-core dataflow (feature-major "transposed" layouts throughout):
  xT [E, N] (f16)  x  wqkvT slices -> q,k as [feat, tok], v as [tok, feat]
  scoresT [ktok, qtok] = k_h^T-chunks x q_h   (PE row-tiled, 2 heads/pass)
  probsT = exp(scoresT/8) in f16 (ACT), causal triangle masked (DVE)
  attnT [feat, qtok] += v-chunk^T x probsT    (PE col-tiled, 2 heads/pass)
  denom[q] += ones^T x probsT                 (PE col-tiled M=1, 4 heads)
  attnT normalized by PE-broadcast reciprocal; out = attnT^T x woT chunks.
Causal structure skips all fully-masked k-blocks (half the attention
flops); diagonal blocks are computed on their valid q-range only.
"""

import os
import sys
import types

import numpy as np

B, N, E, H, D = 2, 2048, 1024, 16, 64
NCORES = 8


# ---------------------------------------------------------------------------
# Environment patches (this container's walrus accepts only one sync wait per
# instruction; the image's antenv lacks the NTFF profile hook shim).
# ---------------------------------------------------------------------------

def _patch_tile_drain():
    import concourse.mybir as mybir
    import concourse.tile as tile_mod
    from concourse.vector_clock import ScopedClock

    if getattr(tile_mod.TileContext, "_drain_patched", False):
        return

    def _drain_and_barrier(self, tick_clock, wait_clock):
        nc = self.nc
        probe = nc.sync.nop()
        wait_clock.add_sem_waits(probe.ins, ScopedClock({None: tick_clock.global_clock}))
        si = probe.ins.sync_info
        waits = list(si.on_wait) if si and si.on_wait else []
        if len(waits) > 1:
            si.on_wait = waits[:1]
            for w in waits[1:]:
                extra = nc.sync.nop()
                extra.ins.sync_info = mybir.SyncInfo(on_wait=[w], on_update=[])
        nc.sync.drain()
        nc.all_engine_barrier()
        assert self.sems is not None
        popped = nc._tile_sem_poison_stack.pop()
        assert popped is self._sem_poison
        nc.clear_and_free_semaphores(list(self.sems.allocated().values()))
        nc.all_engine_barrier()

    tile_mod.TileContext._drain_and_barrier = _drain_and_barrier
    tile_mod.TileContext._drain_patched = True


def _split_sync_waits(nc, max_waits=1):
    import concourse.mybir as mybir

    cnt = 0
    for f in nc.m.functions:
        for blk in f.blocks:
            insts = blk.instructions
            new = []
            for inst in insts:
                si = inst.sync_info
                waits = list(si.on_wait) if si and si.on_wait else []
                if len(waits) > max_waits:
                    keep = waits[-max_waits:]
                    excess = waits[:-max_waits]
                    for j in range(0, len(excess), max_waits):
                        n = mybir.InstNoOp(name=f"I-ws{cnt}", ins=[], outs=[])
                        cnt += 1
                        n.engine = inst.engine
                        n.sync_info = mybir.SyncInfo(
                            on_wait=excess[j:j + max_waits], on_update=[])
                        new.append(n)
                    si.on_wait = keep
                new.append(inst)
            insts[:] = new
    return cnt


def _install_ntff_shim():
    try:
        import antenv
        if "antenv.axon_hooks" in sys.modules:
            return
        mod = types.ModuleType("antenv.axon_hooks")
        mod._hook = None
        mod.set_axon_ntff_profile_hook = lambda h: setattr(mod, "_hook", h)
        mod.get_axon_ntff_profile_hook = lambda: mod._hook
        sys.modules["antenv.axon_hooks"] = mod
        antenv.axon_hooks = mod
        from trn_agent_boot.trn_boot import _ntff_profile_via_ctypes
        mod._hook = _ntff_profile_via_ctypes("/opt/axon/libaxon_pjrt.so")
    except Exception:
        pass


# ---------------------------------------------------------------------------
# Device program (identical on all 8 cores)
# ---------------------------------------------------------------------------

def _build_nc():
    import concourse.bass as bass
    import concourse.mybir as mybir
    import concourse.tile as tile

    _patch_tile_drain()

    f32 = mybir.dt.float32
    f16 = mybir.dt.float16
    AF = mybir.ActivationFunctionType

    nc = bass.Bass("TRN2", target_bir_lowering=False, debug=False)

    xT = nc.dram_tensor("xT", [E, N], f16, kind="ExternalInput")
    wqkvT = nc.dram_tensor("wqkvT", [E, 768], f16, kind="ExternalInput")
    woT = nc.dram_tensor("woT", [256, E], f16, kind="ExternalInput")
    bqkv = nc.dram_tensor("bqkv", [768, 1], f32, kind="ExternalInput")
    tri = nc.dram_tensor("tri", [128, 2, 128], f16, kind="ExternalInput")
    outp = nc.dram_tensor("outp", [N, E], f16, kind="ExternalOutput")

    NB = N // 512          # 4 token blocks of 512
    NT = N // 128          # 16 token tiles of 128
    NE = E // 128          # 8 contraction chunks
    SCALE = float(D) ** -0.5

    with nc.allow_low_precision(reason="fp16 matmul pipeline"), \
            tile.TileContext(nc) as tc:
        with tc.tile_pool(name="const", bufs=1) as constp, \
                tc.tile_pool(name="qk", bufs=1) as qkp, \
                tc.tile_pool(name="probs", bufs=6) as pbp, \
                tc.tile_pool(name="misc", bufs=2) as miscp, \
                tc.tile_pool(name="stage", bufs=4) as stp, \
                tc.tile_pool(name="mm", bufs=1, space="PSUM") as mmp, \
                tc.tile_pool(name="sc", bufs=2, space="PSUM") as scp, \
                tc.tile_pool(name="pv", bufs=1, space="PSUM") as pvp, \
                tc.tile_pool(name="den", bufs=1, space="PSUM") as denp:

            xT_sb = constp.tile([128, NE, N], f16, tag="xT")
            wq_sb = constp.tile([128, NE, 768], f16, tag="wq")
            wo_sb = constp.tile([128, 2, E], f16, tag="wo")
            bias_sb = constp.tile([128, 6, 1], f32, tag="bias")
            tri_sb = constp.tile([128, 2, 128], f16, tag="tri")
            ones_sb = constp.tile([128, 128], f16, tag="ones")
            q_sb = qkp.tile([128, 2, N], f16, tag="q")
            k_sb = qkp.tile([128, 2, N], f16, tag="k")
            vt_sb = qkp.tile([128, NT, 256], f16, tag="vt")
            at_sb = qkp.tile([128, 2, N], f16, tag="at")

            nc.vector.memset(ones_sb[:], 1.0)
            # ACT table primer: load the exp/ln table set during the DMA ramp
            # (first real exp would otherwise pay ~2.7us mid-pipeline).
            nc.scalar.activation(at_sb[:, 0, 0:8], ones_sb[:, 0:8],
                                 AF.Exp, scale=1.0)
            nc.scalar.activation(at_sb[:, 0, 8:16], ones_sb[:, 8:16], AF.Ln)
            # PE warm-up spinner: gapless full-array accumulation chains keep
            # the PE HAM monitor busy while inputs stream in, so real matmuls
            # start at 2.4 GHz instead of paying the cold-clock (1.2 GHz)
            # penalty.  The HAM only un-throttles after ~3.4us of PE activity
            # with no gaps, so mimic the qkv unit structure (8-matmul
            # accumulation chains).
            for s in range(6):
                sp = mmp.tile([128, 512], f32, tag="mm", name=f"spin{s}")
                for e in range(8):
                    nc.tensor.matmul(sp[:, 0:128], ones_sb[:],
                                     q_sb[:, 0, 0:128],
                                     start=(e == 0), stop=(e == 7))
            # x on the sync queue, token-block 0 first (first qk unit needs it)
            for nb in range(4):
                for i in range(NE):
                    nc.sync.dma_start(
                        xT_sb[:, i, nb * 512:(nb + 1) * 512],
                        xT.ap()[i * 128:(i + 1) * 128, nb * 512:(nb + 1) * 512])
            # qkv weights on the gpsimd queue (runs concurrently with x);
            # scalar issues no DMA at all -- it is the exp bottleneck engine.
            for i in range(NE):
                nc.gpsimd.dma_start(wq_sb[:, i, :], wqkvT.ap()[i * 128:(i + 1) * 128, :])
            # small loads on scalar: ~3us of DMA, done before its first exp
            for i in range(6):
                nc.scalar.dma_start(bias_sb[:, i, :], bqkv.ap()[i * 128:(i + 1) * 128, :])
            nc.scalar.dma_start(tri_sb[:], tri.ap())
            for i in range(2):
                nc.scalar.dma_start(wo_sb[:, i, :], woT.ap()[i * 128:(i + 1) * 128, :])

            # -- filler work units (emitted interleaved into attention) ----
            # Pre-loop fillers rotate over the 4 single-bank psum pools that
            # are free before the j-loop allocates them, so unit n+1's
            # matmuls never wait on unit n's DVE evacuation (WAR on the
            # single mm buffer).
            _prepools = [(mmp, "mm"), (pvp, "pv0"), (pvp, "pv1"),
                         (denp, "den")]
            _prectr = [0]

            def _filler_ps(name, pre):
                # always [128, 512] so pool slots stay consistently sized
                if pre:
                    pool, tag = _prepools[_prectr[0] % 4]
                    _prectr[0] += 1
                    return pool.tile([128, 512], f32, tag=tag, name=name)
                return mmp.tile([128, 512], f32, tag="mm", name=name)

            def emit_qk_unit(ft, nb, pre=False):
                # q/k feature tile ft (0,1=q; 2,3=k), token block nb
                ps = _filler_ps(f"qk_{ft}_{nb}", pre)
                for e in range(NE):
                    nc.tensor.matmul(
                        ps[:], wq_sb[:, e, ft * 128:(ft + 1) * 128],
                        xT_sb[:, e, nb * 512:(nb + 1) * 512],
                        start=(e == 0), stop=(e == NE - 1))
                dest = (q_sb if ft < 2 else k_sb)[:, ft % 2,
                                                  nb * 512:(nb + 1) * 512]
                nc.vector.tensor_scalar_add(dest, ps[:], bias_sb[:, ft, :])

            def emit_v_unit(tt, pre=False):
                ps = _filler_ps(f"v_{tt}", pre)
                for e in range(NE):
                    nc.tensor.matmul(
                        ps[:, 0:256], xT_sb[:, e, tt * 128:(tt + 1) * 128],
                        wq_sb[:, e, 512:768],
                        start=(e == 0), stop=(e == NE - 1))
                nc.vector.tensor_copy(vt_sb[:, tt, :], ps[:, 0:256])

            store_ctr = [0]

            def emit_store(ap_out, st):
                # round-robin output stores over sync/sync/gpsimd queues
                eng = nc.gpsimd if store_ctr[0] % 3 == 2 else nc.sync
                store_ctr[0] += 1
                eng.dma_start(ap_out, st)

            def emit_p3_unit(tt):
                for n2 in range(2):
                    ps = mmp.tile([128, 512], f32, tag="mm",
                                  name=f"p3_{tt}_{n2}")
                    for fp in range(2):
                        nc.tensor.matmul(
                            ps[:],
                            at_sb[:, fp, tt * 128:(tt + 1) * 128],
                            wo_sb[:, fp, n2 * 512:(n2 + 1) * 512],
                            start=(fp == 0), stop=(fp == 1))
                    st = stp.tile([128, 512], f16, tag="st",
                                  name=f"st_{tt}_{n2}")
                    nc.vector.tensor_copy(st[:], ps[:])
                    emit_store(
                        outp.ap()[tt * 128:(tt + 1) * 128,
                                  n2 * 512:(n2 + 1) * 512], st[:])

            def emit_norm_b(item, tail=False):
                # PE broadcast of 1/denom + normalize into attnT (+v bias)
                jj, araw, rec = item
                js = slice(512 * jj, 512 * (jj + 1))
                for p in (0, 1):
                    if tail:
                        # scores pool is free at the tail: its 2 buffers let
                        # bc(p1) run without serializing behind mul(p0) on
                        # the single-buffer mm pool
                        bct = scp.tile([128, 2, 512], f32, tag="sc",
                                       name=f"bc_{jj}_{p}")
                        bc_hh = lambda hh: bct[64 * hh:64 * hh + 64, 0, :]
                        bc_all = bct[:, 0, :]
                    else:
                        bcm = mmp.tile([128, 512], f32, tag="mm",
                                       name=f"bc_{jj}_{p}")
                        bc_hh = lambda hh: bcm[64 * hh:64 * hh + 64, :]
                        bc_all = bcm[:]
                    for hh in (0, 1):
                        h = 2 * p + hh
                        nc.tensor.matmul(
                            bc_hh(hh),
                            ones_sb[32 * h:32 * h + 1, 0:64],
                            rec[32 * h:32 * h + 1, :],
                            start=True, stop=True,
                            tile_position=(32 * h, 64 * hh))
                    nc.vector.tensor_mul(at_sb[:, p, js], araw[p][:], bc_all)
                    nc.vector.tensor_scalar_add(at_sb[:, p, js],
                                                at_sb[:, p, js],
                                                bias_sb[:, 4 + p, :])

            # -- p1 block 0 up front (attention j=0 needs it) --------------
            for ft in range(4):
                emit_qk_unit(ft, 0, pre=True)
            for tt in range(4):
                emit_v_unit(tt, pre=True)

            # -- attention blocks: depth-2 software pipeline ---------------
            # scores/exp for k-block ik+2 are emitted while pv/den of block
            # ik execute, so ACT (exp) and PE overlap instead of ping-pong.
            pending = None
            for j in range(NB):
                if j == 0:
                    fillers = [(emit_qk_unit, (ft, 1)) for ft in range(4)]
                    fillers += [(emit_v_unit, (tt,)) for tt in range(4, 8)]
                elif j == 1:
                    fillers = [(emit_qk_unit, (ft, 2)) for ft in range(4)]
                    fillers += [(emit_v_unit, (tt,)) for tt in range(8, 12)]
                elif j == 2:
                    fillers = [(emit_qk_unit, (ft, 3)) for ft in range(4)]
                    fillers += [(emit_v_unit, (tt,)) for tt in range(12, 16)]
                    fillers += [(emit_p3_unit, (tt,)) for tt in range(4)]
                else:
                    fillers = [(emit_p3_unit, (tt,)) for tt in range(4, 12)]
                if pending is not None:
                    fillers.insert(min(2, len(fillers)), (emit_norm_b, (pending,)))
                    pending = None
                nf = len(fillers)
                pv_ps = [pvp.tile([128, 512], f32, tag=f"pv{p}",
                                  name=f"pv{p}_{j}") for p in (0, 1)]
                den_ps = denp.tile([128, 512], f32, tag="den",
                                   name=f"den_{j}")
                nk = 4 * (j + 1)

                def emit_scores(ik, j=j):
                    r = ik - 4 * j
                    qoff = 128 * r if r > 0 else 0
                    qs = slice(512 * j + qoff, 512 * (j + 1))
                    pbs = []
                    for p in (0, 1):
                        sc = scp.tile([128, 2, 512], f32, tag="sc",
                                      name=f"sc_{j}_{ik}_{p}")
                        for hh in (0, 1):
                            dsl = slice(64 * hh, 64 * hh + 64)
                            nc.tensor.matmul(
                                sc[:, hh, qoff:512],
                                k_sb[dsl, p, ik * 128:(ik + 1) * 128],
                                q_sb[dsl, p, qs],
                                start=True, stop=True)
                        pb = pbp.tile([128, 2, 512], f16, tag="pb",
                                      name=f"pb_{j}_{ik}_{p}")
                        nc.scalar.activation(pb[:, :, qoff:512],
                                             sc[:, :, qoff:512],
                                             AF.Exp, scale=SCALE)
                        if r >= 0:
                            nc.gpsimd.tensor_mul(
                                pb[:, :, qoff:qoff + 128],
                                pb[:, :, qoff:qoff + 128], tri_sb[:])
                        pbs.append(pb)
                    return pbs

                stage = {0: emit_scores(0)}
                if nk > 1:
                    stage[1] = emit_scores(1)
                fdone = 0
                for ik in range(nk):
                    r = ik - 4 * j
                    qoff = 128 * r if r > 0 else 0
                    first, last = ik == 0, ik == nk - 1
                    pbs = stage.pop(ik)
                    for p in (0, 1):
                        for hh in (0, 1):
                            h = 2 * p + hh
                            nc.tensor.matmul(
                                pv_ps[p][64 * hh:64 * hh + 64, qoff:512],
                                vt_sb[:, ik, 64 * h:64 * h + 64],
                                pbs[p][:, hh, qoff:512],
                                start=first, stop=last,
                                tile_position=(0, 64 * hh),
                                skip_group_check=True)
                    for h in range(4):
                        nc.tensor.matmul(
                            den_ps[32 * h:32 * h + 1, qoff:512],
                            ones_sb[:, 0:1],
                            pbs[h // 2][:, h % 2, qoff:512],
                            start=first, stop=last,
                            tile_position=(0, 32 * h),
                            skip_group_check=True)
                    want = ((ik + 1) * nf) // nk
                    # spread fillers around the scores emission so attention
                    # matmuls sit between consecutive fillers (hides the
                    # filler psum-evacuation latency on the shared mm bank)
                    if fdone < want:
                        fn, args = fillers[fdone]
                        fn(*args)
                        fdone += 1
                    if ik + 2 < nk:
                        stage[ik + 2] = emit_scores(ik + 2)
                    while fdone < want:
                        fn, args = fillers[fdone]
                        fn(*args)
                        fdone += 1
                # den copy first so the ACT reciprocal chain starts early
                den_sb = miscp.tile([128, 512], f32, tag="densb",
                                    name=f"densb_{j}")
                nc.vector.tensor_copy(den_sb[0:97, :], den_ps[0:97, :])
                araw = []
                for p in (0, 1):
                    ar = miscp.tile([128, 512], f32, tag=f"araw{p}",
                                    name=f"araw{p}_{j}")
                    nc.vector.tensor_copy(ar[:], pv_ps[p][:])
                    araw.append(ar)
                rec = miscp.tile([128, 512], f16, tag="rec", name=f"rec_{j}")
                # 1/x = exp(-ln x) on ACT (2x0.72us).  The DVE iterative
                # reciprocal takes 3.4us and, worse, blocks the in-order DVE
                # queue that the filler psum evacuations ride on -- measured
                # 2.3us PE stalls at every j boundary from exactly that.
                lnden = miscp.tile([128, 512], f32, tag="lnden",
                                   name=f"lnden_{j}")
                nc.scalar.activation(lnden[0:97, :], den_sb[0:97, :], AF.Ln)
                nc.scalar.activation(rec[0:97, :], lnden[0:97, :],
                                     AF.Exp, scale=-1.0)
                pending = (j, araw, rec)

            # -- tail: final normalize + last output tiles -----------------
            emit_norm_b(pending, tail=True)
            for idx, tt in enumerate(range(12, NT)):
                for n2 in range(2):
                    # rotate psum across the (now free) sc + mm + pv pools
                    # so matmuls are not gated on the previous tile's cast
                    k = (2 * idx + n2) % 4
                    if k < 2:
                        pst = scp.tile([128, 2, 512], f32, tag="sc",
                                       name=f"p3t_{tt}_{n2}")
                        ps = pst[:, 0, :]
                    elif k == 2:
                        ps = mmp.tile([128, 512], f32, tag="mm",
                                      name=f"p3t_{tt}_{n2}")[:]
                    else:
                        ps = pvp.tile([128, 512], f32, tag="pv0",
                                      name=f"p3t_{tt}_{n2}")[:]
                    for fp in range(2):
                        nc.tensor.matmul(
                            ps,
                            at_sb[:, fp, tt * 128:(tt + 1) * 128],
                            wo_sb[:, fp, n2 * 512:(n2 + 1) * 512],
                            start=(fp == 0), stop=(fp == 1))
                    st = stp.tile([128, 512], f16, tag="st",
                                  name=f"stt_{tt}_{n2}")
                    # split tail psum evacuation across both cast engines
                    # (scalar is exp-free by now)
                    ceng = nc.scalar if (2 * idx + n2) % 2 else nc.vector
                    if ceng is nc.scalar:
                        nc.scalar.activation(st[:], ps, AF.Copy)
                    else:
                        nc.vector.tensor_copy(st[:], ps)
                    emit_store(
                        outp.ap()[tt * 128:(tt + 1) * 128,
                                  n2 * 512:(n2 + 1) * 512], st[:])

    _split_sync_waits(nc)
    return nc


_NC = None


def _get_nc():
    global _NC
    if _NC is None:
        _NC = _build_nc()
    return _NC


# ---------------------------------------------------------------------------
# Host entry point
# ---------------------------------------------------------------------------

def kernel(x, qkv_w, qkv_b, out_w, out_b):
    from concourse.bass_utils import run_bass_kernel_spmd

    trace_dir = os.environ.get("BASS_KERNEL_TRACE_DIR")
    if trace_dir:
        _install_ntff_shim()

    nc = _get_nc()

    x = np.asarray(x, np.float32)
    qkv_w = np.asarray(qkv_w, np.float32)
    qkv_b = np.asarray(qkv_b, np.float32)
    out_w = np.asarray(out_w, np.float32)
    out_b = np.asarray(out_b, np.float32)

    tri_np = np.broadcast_to(np.triu(np.ones((128, 128), np.float16))[:, None, :],
        (128, 2, 128)).copy()
    in_maps = []
    for c in range(NCORES):
        b, g = divmod(c, 4)
        fs = slice(256 * g, 256 * g + 256)
        wqkvT = np.ascontiguousarray(
            np.concatenate([qkv_w[0 * E:1 * E][fs],
                            qkv_w[1 * E:2 * E][fs],
                            qkv_w[2 * E:3 * E][fs]], axis=0).T)
        bq = np.concatenate([qkv_b[0 * E:1 * E][fs],
                             qkv_b[1 * E:2 * E][fs],
                             qkv_b[2 * E:3 * E][fs]])[:, None]
        in_maps.append({
            "xT": np.ascontiguousarray(x[b].T).astype(np.float16),
            "wqkvT": wqkvT.astype(np.float16),
            "woT": np.ascontiguousarray(out_w[:, fs].T).astype(np.float16),
            "bqkv": np.ascontiguousarray(bq),
            "tri": tri_np,
        })

    kwargs = {}
    if trace_dir:
        kwargs = {"trace": True, "tmpdir": trace_dir}
    res = run_bass_kernel_spmd(nc, in_maps, core_ids=list(range(NCORES)), **kwargs)
    if trace_dir and res.exec_time_ns is not None:
        print(f"HW exec time: {res.exec_time_ns} ns")

    out = np.zeros((B, N, E), np.float32)
    for c in range(NCORES):
        out[c // 4] += res.results[c]["outp"].astype(np.float32)
    out += out_b[None, None, :]
    return out



# revision 21
# speedup vs baseline: 1.1499x; 1.0259x over previous
"""Causal self-attention (B=2, N=2048, E=1024, H=16, D=64) on 8 TRN2 cores.

Sharding: core c -> batch b = c//4, head group g = c%4 (4 heads = 256
features per core).  Each core computes its heads' q/k/v projections,
causal attention, and a partial out-projection; the host sums the 4
partials per batch.

Per-core dataflow (feature-major "transposed" layouts throughout):
  xT [E, N] (f16)  x  wqkvT slices -> q,k as [feat, tok], v as [tok, feat]
  scoresT [ktok, qtok] = k_h^T-chunks x q_h   (PE row-tiled, 2 heads/pass)
  probsT = exp(scoresT/8) in f16 (ACT), causal triangle masked (DVE)
  attnT [feat, qtok] += v-chunk^T x probsT    (PE col-tiled, 2 heads/pass)
  denom[q] += ones^T x probsT                 (PE col-tiled M=1, 4 heads)
  attnT normalized by PE-broadcast reciprocal; out = attnT^T x woT chunks.
Causal structure skips all fully-masked k-blocks (half the attention
flops); diagonal blocks are computed on their valid q-range only.
"""

import os
import sys
import types

import numpy as np

B, N, E, H, D = 2, 2048, 1024, 16, 64
NCORES = 8


# ---------------------------------------------------------------------------
# Environment patches (this container's walrus accepts only one sync wait per
# instruction; the image's antenv lacks the NTFF profile hook shim).
# ---------------------------------------------------------------------------

def _patch_tile_drain():
    import concourse.mybir as mybir
    import concourse.tile as tile_mod
    from concourse.vector_clock import ScopedClock

    if getattr(tile_mod.TileContext, "_drain_patched", False):
        return

    def _drain_and_barrier(self, tick_clock, wait_clock):
        nc = self.nc
        probe = nc.sync.nop()
        wait_clock.add_sem_waits(probe.ins, ScopedClock({None: tick_clock.global_clock}))
        si = probe.ins.sync_info
        waits = list(si.on_wait) if si and si.on_wait else []
        if len(waits) > 1:
            si.on_wait = waits[:1]
            for w in waits[1:]:
                extra = nc.sync.nop()
                extra.ins.sync_info = mybir.SyncInfo(on_wait=[w], on_update=[])
        nc.sync.drain()
        nc.all_engine_barrier()
        assert self.sems is not None
        popped = nc._tile_sem_poison_stack.pop()
        assert popped is self._sem_poison
        nc.clear_and_free_semaphores(list(self.sems.allocated().values()))
        nc.all_engine_barrier()

    tile_mod.TileContext._drain_and_barrier = _drain_and_barrier
    tile_mod.TileContext._drain_patched = True


def _split_sync_waits(nc, max_waits=1):
    import concourse.mybir as mybir

    cnt = 0
    for f in nc.m.functions:
        for blk in f.blocks:
            insts = blk.instructions
            new = []
            for inst in insts:
                si = inst.sync_info
                waits = list(si.on_wait) if si and si.on_wait else []
                if len(waits) > max_waits:
                    keep = waits[-max_waits:]
                    excess = waits[:-max_waits]
                    for j in range(0, len(excess), max_waits):
                        n = mybir.InstNoOp(name=f"I-ws{cnt}", ins=[], outs=[])
                        cnt += 1
                        n.engine = inst.engine
                        n.sync_info = mybir.SyncInfo(
                            on_wait=excess[j:j + max_waits], on_update=[])
                        new.append(n)
                    si.on_wait = keep
                new.append(inst)
            insts[:] = new
    return cnt


def _install_ntff_shim():
    try:
        import antenv
        if "antenv.axon_hooks" in sys.modules:
            return
        mod = types.ModuleType("antenv.axon_hooks")
        mod._hook = None
        mod.set_axon_ntff_profile_hook = lambda h: setattr(mod, "_hook", h)
        mod.get_axon_ntff_profile_hook = lambda: mod._hook
        sys.modules["antenv.axon_hooks"] = mod
        antenv.axon_hooks = mod
        from trn_agent_boot.trn_boot import _ntff_profile_via_ctypes
        mod._hook = _ntff_profile_via_ctypes("/opt/axon/libaxon_pjrt.so")
    except Exception:
        pass


# ---------------------------------------------------------------------------
# Device program (identical on all 8 cores)
# ---------------------------------------------------------------------------

def _build_nc():
    import concourse.bass as bass
    import concourse.mybir as mybir
    import concourse.tile as tile

    _patch_tile_drain()

    f32 = mybir.dt.float32
    f16 = mybir.dt.float16
    AF = mybir.ActivationFunctionType

    nc = bass.Bass("TRN2", target_bir_lowering=False, debug=False)

    xT = nc.dram_tensor("xT", [E, N], f16, kind="ExternalInput")
    wqkvT = nc.dram_tensor("wqkvT", [E, 768], f16, kind="ExternalInput")
    woT = nc.dram_tensor("woT", [256, E], f16, kind="ExternalInput")
    bqkv = nc.dram_tensor("bqkv", [768, 1], f32, kind="ExternalInput")
    tri = nc.dram_tensor("tri", [128, 2, 128], f16, kind="ExternalInput")
    outp = nc.dram_tensor("outp", [N, E], f16, kind="ExternalOutput")

    NB = N // 512          # 4 token blocks of 512
    NT = N // 128          # 16 token tiles of 128
    NE = E // 128          # 8 contraction chunks
    SCALE = float(D) ** -0.5

    with nc.allow_low_precision(reason="fp16 matmul pipeline"), \
            tile.TileContext(nc) as tc:
        with tc.tile_pool(name="const", bufs=1) as constp, \
                tc.tile_pool(name="qk", bufs=1) as qkp, \
                tc.tile_pool(name="probs", bufs=6) as pbp, \
                tc.tile_pool(name="misc", bufs=2) as miscp, \
                tc.tile_pool(name="stage", bufs=4) as stp, \
                tc.tile_pool(name="mm", bufs=1, space="PSUM") as mmp, \
                tc.tile_pool(name="sc", bufs=2, space="PSUM") as scp, \
                tc.tile_pool(name="pv", bufs=1, space="PSUM") as pvp, \
                tc.tile_pool(name="den", bufs=1, space="PSUM") as denp:

            xT_sb = constp.tile([128, NE, N], f16, tag="xT")
            wq_sb = constp.tile([128, NE, 768], f16, tag="wq")
            wo_sb = constp.tile([128, 2, E], f16, tag="wo")
            bias_sb = constp.tile([128, 6, 1], f32, tag="bias")
            tri_sb = constp.tile([128, 2, 128], f16, tag="tri")
            ones_sb = constp.tile([128, 128], f16, tag="ones")
            q_sb = qkp.tile([128, 2, N], f16, tag="q")
            k_sb = qkp.tile([128, 2, N], f16, tag="k")
            vt_sb = qkp.tile([128, NT, 256], f16, tag="vt")
            at_sb = qkp.tile([128, 2, N], f16, tag="at")

            nc.vector.memset(ones_sb[:], 1.0)
            # ACT table primer: load the exp/ln table set during the DMA ramp
            # (first real exp would otherwise pay ~2.7us mid-pipeline).
            nc.scalar.activation(at_sb[:, 0, 0:8], ones_sb[:, 0:8],
                                 AF.Exp, scale=1.0)
            nc.scalar.activation(at_sb[:, 0, 8:16], ones_sb[:, 8:16], AF.Ln)
            # PE warm-up spinner: gapless full-array accumulation chains keep
            # the PE HAM monitor busy while inputs stream in, so real matmuls
            # start at 2.4 GHz instead of paying the cold-clock (1.2 GHz)
            # penalty.  The HAM only un-throttles after ~3.4us of PE activity
            # with no gaps, so mimic the qkv unit structure (8-matmul
            # accumulation chains).
            for s in range(6):
                sp = mmp.tile([128, 512], f32, tag="mm", name=f"spin{s}")
                for e in range(8):
                    nc.tensor.matmul(sp[:, 0:128], ones_sb[:],
                                     q_sb[:, 0, 0:128],
                                     start=(e == 0), stop=(e == 7))
            # x on the sync queue, token-block 0 first (first qk unit needs it)
            for nb in range(4):
                for i in range(NE):
                    nc.sync.dma_start(
                        xT_sb[:, i, nb * 512:(nb + 1) * 512],
                        xT.ap()[i * 128:(i + 1) * 128, nb * 512:(nb + 1) * 512])
            # qkv weights on the gpsimd queue (runs concurrently with x);
            # scalar issues no DMA at all -- it is the exp bottleneck engine.
            for i in range(NE):
                nc.gpsimd.dma_start(wq_sb[:, i, :], wqkvT.ap()[i * 128:(i + 1) * 128, :])
            # small loads on scalar: ~3us of DMA, done before its first exp
            for i in range(6):
                nc.scalar.dma_start(bias_sb[:, i, :], bqkv.ap()[i * 128:(i + 1) * 128, :])
            nc.scalar.dma_start(tri_sb[:], tri.ap())
            for i in range(2):
                nc.scalar.dma_start(wo_sb[:, i, :], woT.ap()[i * 128:(i + 1) * 128, :])

            # -- filler work units (emitted interleaved into attention) ----
            # Pre-loop fillers rotate over the 4 single-bank psum pools that
            # are free before the j-loop allocates them, so unit n+1's
            # matmuls never wait on unit n's DVE evacuation (WAR on the
            # single mm buffer).
            _prepools = [(mmp, "mm"), (pvp, "pv0"), (pvp, "pv1"),
                         (denp, "den")]
            _prectr = [0]

            def _filler_ps(name, pre):
                # always [128, 512] so pool slots stay consistently sized
                if pre:
                    pool, tag = _prepools[_prectr[0] % 4]
                    _prectr[0] += 1
                    return pool.tile([128, 512], f32, tag=tag, name=name)
                return mmp.tile([128, 512], f32, tag="mm", name=name)

            def emit_qk_unit(ft, nb, pre=False):
                # q/k feature tile ft (0,1=q; 2,3=k), token block nb
                ps = _filler_ps(f"qk_{ft}_{nb}", pre)
                for e in range(NE):
                    nc.tensor.matmul(
                        ps[:], wq_sb[:, e, ft * 128:(ft + 1) * 128],
                        xT_sb[:, e, nb * 512:(nb + 1) * 512],
                        start=(e == 0), stop=(e == NE - 1))
                dest = (q_sb if ft < 2 else k_sb)[:, ft % 2,
                                                  nb * 512:(nb + 1) * 512]
                nc.vector.tensor_scalar_add(dest, ps[:], bias_sb[:, ft, :])

            def emit_v_unit(tt, pre=False):
                ps = _filler_ps(f"v_{tt}", pre)
                for e in range(NE):
                    nc.tensor.matmul(
                        ps[:, 0:256], xT_sb[:, e, tt * 128:(tt + 1) * 128],
                        wq_sb[:, e, 512:768],
                        start=(e == 0), stop=(e == NE - 1))
                nc.vector.tensor_copy(vt_sb[:, tt, :], ps[:, 0:256])

            store_ctr = [0]

            def emit_store(ap_out, st, tail=False):
                # round-robin output stores; strict alternation at the tail
                # so the last few stores drain on two queues in parallel
                if tail:
                    eng = nc.gpsimd if store_ctr[0] % 2 else nc.sync
                else:
                    eng = nc.gpsimd if store_ctr[0] % 3 == 2 else nc.sync
                store_ctr[0] += 1
                eng.dma_start(ap_out, st)

            def emit_p3_unit(tt, scalar_cast=False):
                for n2 in range(2):
                    ps = mmp.tile([128, 512], f32, tag="mm",
                                  name=f"p3_{tt}_{n2}")
                    for fp in range(2):
                        nc.tensor.matmul(
                            ps[:],
                            at_sb[:, fp, tt * 128:(tt + 1) * 128],
                            wo_sb[:, fp, n2 * 512:(n2 + 1) * 512],
                            start=(fp == 0), stop=(fp == 1))
                    st = stp.tile([128, 512], f16, tag="st",
                                  name=f"st_{tt}_{n2}")
                    if scalar_cast:
                        # keep the DVE queue clear for the j=3 norm chain
                        nc.scalar.activation(st[:], ps[:], AF.Copy)
                    else:
                        nc.vector.tensor_copy(st[:], ps[:])
                    emit_store(
                        outp.ap()[tt * 128:(tt + 1) * 128,
                                  n2 * 512:(n2 + 1) * 512], st[:])

            def emit_norm_b(item, tail=False):
                # PE broadcast of 1/denom + normalize into attnT (+v bias)
                jj, araw, rec = item
                js = slice(512 * jj, 512 * (jj + 1))
                for p in (0, 1):
                    if tail:
                        # scores pool is free at the tail: its 2 buffers let
                        # bc(p1) run without serializing behind mul(p0) on
                        # the single-buffer mm pool
                        bct = scp.tile([128, 2, 512], f32, tag="sc",
                                       name=f"bc_{jj}_{p}")
                        bc_hh = lambda hh: bct[64 * hh:64 * hh + 64, 0, :]
                        bc_all = bct[:, 0, :]
                    else:
                        bcm = mmp.tile([128, 512], f32, tag="mm",
                                       name=f"bc_{jj}_{p}")
                        bc_hh = lambda hh: bcm[64 * hh:64 * hh + 64, :]
                        bc_all = bcm[:]
                    for hh in (0, 1):
                        h = 2 * p + hh
                        nc.tensor.matmul(
                            bc_hh(hh),
                            ones_sb[32 * h:32 * h + 1, 0:64],
                            rec[32 * h:32 * h + 1, :],
                            start=True, stop=True,
                            tile_position=(32 * h, 64 * hh))
                    nc.vector.tensor_mul(at_sb[:, p, js], araw[p][:], bc_all)
                    nc.vector.tensor_scalar_add(at_sb[:, p, js],
                                                at_sb[:, p, js],
                                                bias_sb[:, 4 + p, :])

            # -- p1 block 0 up front (attention j=0 needs it) --------------
            for ft in range(4):
                emit_qk_unit(ft, 0, pre=True)
            for tt in range(4):
                emit_v_unit(tt, pre=True)

            # -- attention blocks: depth-2 software pipeline ---------------
            # scores/exp for k-block ik+2 are emitted while pv/den of block
            # ik execute, so ACT (exp) and PE overlap instead of ping-pong.
            # Filler placement balances each j-phase's PE deficit against
            # its ACT (exp) load: late j's are exp-heavy, so projection/
            # out-proj work migrates as late as dependencies allow.
            pending = None
            for j in range(NB):
                if j == 0:
                    fillers = [(emit_qk_unit, (ft, 1)) for ft in range(4)]
                elif j == 1:
                    fillers = [(emit_v_unit, (tt,)) for tt in range(4, 8)]
                    fillers += [(emit_qk_unit, (ft, 2)) for ft in range(4)]
                elif j == 2:
                    fillers = [(emit_v_unit, (tt,)) for tt in range(8, 12)]
                    fillers += [(emit_qk_unit, (ft, 3)) for ft in range(4)]
                    fillers += [(emit_p3_unit, (tt,)) for tt in range(2)]
                else:
                    fillers = [(emit_v_unit, (tt,)) for tt in range(12, 16)]
                    fillers += [(emit_p3_unit, (tt,)) for tt in range(2, 10)]
                    fillers += [(emit_p3_unit, (tt, True)) for tt in (10, 11)]
                if pending is not None:
                    fillers.insert(min(2, len(fillers)), (emit_norm_b, (pending,)))
                    pending = None
                nf = len(fillers)
                pv_ps = [pvp.tile([128, 512], f32, tag=f"pv{p}",
                                  name=f"pv{p}_{j}") for p in (0, 1)]
                den_ps = denp.tile([128, 512], f32, tag="den",
                                   name=f"den_{j}")
                nk = 4 * (j + 1)

                def emit_scores(ik, j=j):
                    r = ik - 4 * j
                    qoff = 128 * r if r > 0 else 0
                    qs = slice(512 * j + qoff, 512 * (j + 1))
                    pbs = []
                    for p in (0, 1):
                        sc = scp.tile([128, 2, 512], f32, tag="sc",
                                      name=f"sc_{j}_{ik}_{p}")
                        for hh in (0, 1):
                            dsl = slice(64 * hh, 64 * hh + 64)
                            nc.tensor.matmul(
                                sc[:, hh, qoff:512],
                                k_sb[dsl, p, ik * 128:(ik + 1) * 128],
                                q_sb[dsl, p, qs],
                                start=True, stop=True)
                        pb = pbp.tile([128, 2, 512], f16, tag="pb",
                                      name=f"pb_{j}_{ik}_{p}")
                        nc.scalar.activation(pb[:, :, qoff:512],
                                             sc[:, :, qoff:512],
                                             AF.Exp, scale=SCALE)
                        if r >= 0:
                            nc.gpsimd.tensor_mul(
                                pb[:, :, qoff:qoff + 128],
                                pb[:, :, qoff:qoff + 128], tri_sb[:])
                        pbs.append(pb)
                    return pbs

                stage = {0: emit_scores(0)}
                if nk > 1:
                    stage[1] = emit_scores(1)
                fdone = 0
                for ik in range(nk):
                    r = ik - 4 * j
                    qoff = 128 * r if r > 0 else 0
                    first, last = ik == 0, ik == nk - 1
                    pbs = stage.pop(ik)
                    for p in (0, 1):
                        for hh in (0, 1):
                            h = 2 * p + hh
                            nc.tensor.matmul(
                                pv_ps[p][64 * hh:64 * hh + 64, qoff:512],
                                vt_sb[:, ik, 64 * h:64 * h + 64],
                                pbs[p][:, hh, qoff:512],
                                start=first, stop=last,
                                tile_position=(0, 64 * hh),
                                skip_group_check=True)
                    for h in range(4):
                        nc.tensor.matmul(
                            den_ps[32 * h:32 * h + 1, qoff:512],
                            ones_sb[:, 0:1],
                            pbs[h // 2][:, h % 2, qoff:512],
                            start=first, stop=last,
                            tile_position=(0, 32 * h),
                            skip_group_check=True)
                    want = ((ik + 1) * nf) // nk
                    # spread fillers around the scores emission so attention
                    # matmuls sit between consecutive fillers (hides the
                    # filler psum-evacuation latency on the shared mm bank)
                    if fdone < want:
                        fn, args = fillers[fdone]
                        fn(*args)
                        fdone += 1
                    if ik + 2 < nk:
                        stage[ik + 2] = emit_scores(ik + 2)
                    while fdone < want:
                        fn, args = fillers[fdone]
                        fn(*args)
                        fdone += 1
                # den copy first so the ACT reciprocal chain starts early
                den_sb = miscp.tile([128, 512], f32, tag="densb",
                                    name=f"densb_{j}")
                nc.vector.tensor_copy(den_sb[0:97, :], den_ps[0:97, :])
                araw = []
                for p in (0, 1):
                    ar = miscp.tile([128, 512], f32, tag=f"araw{p}",
                                    name=f"araw{p}_{j}")
                    nc.vector.tensor_copy(ar[:], pv_ps[p][:])
                    araw.append(ar)
                rec = miscp.tile([128, 512], f16, tag="rec", name=f"rec_{j}")
                # 1/x = exp(-ln x) on ACT (2x0.72us).  The DVE iterative
                # reciprocal takes 3.4us and, worse, blocks the in-order DVE
                # queue that the filler psum evacuations ride on -- measured
                # 2.3us PE stalls at every j boundary from exactly that.
                lnden = miscp.tile([128, 512], f32, tag="lnden",
                                   name=f"lnden_{j}")
                nc.scalar.activation(lnden[0:97, :], den_sb[0:97, :], AF.Ln)
                nc.scalar.activation(rec[0:97, :], lnden[0:97, :],
                                     AF.Exp, scale=-1.0)
                pending = (j, araw, rec)

            # -- tail: final normalize + last output tiles -----------------
            emit_norm_b(pending, tail=True)
            for idx, tt in enumerate(range(12, NT)):
                for n2 in range(2):
                    # rotate psum across the (now free) sc + mm + pv pools
                    # so matmuls are not gated on the previous tile's cast
                    k = (2 * idx + n2) % 4
                    if k < 2:
                        pst = scp.tile([128, 2, 512], f32, tag="sc",
                                       name=f"p3t_{tt}_{n2}")
                        ps = pst[:, 0, :]
                    elif k == 2:
                        ps = mmp.tile([128, 512], f32, tag="mm",
                                      name=f"p3t_{tt}_{n2}")[:]
                    else:
                        ps = pvp.tile([128, 512], f32, tag="pv0",
                                      name=f"p3t_{tt}_{n2}")[:]
                    for fp in range(2):
                        nc.tensor.matmul(
                            ps,
                            at_sb[:, fp, tt * 128:(tt + 1) * 128],
                            wo_sb[:, fp, n2 * 512:(n2 + 1) * 512],
                            start=(fp == 0), stop=(fp == 1))
                    st = stp.tile([128, 512], f16, tag="st",
                                  name=f"stt_{tt}_{n2}")
                    # split tail psum evacuation across both cast engines
                    # (scalar is exp-free by now)
                    ceng = nc.scalar if (2 * idx + n2) % 2 else nc.vector
                    if ceng is nc.scalar:
                        nc.scalar.activation(st[:], ps, AF.Copy)
                    else:
                        nc.vector.tensor_copy(st[:], ps)
                    emit_store(
                        outp.ap()[tt * 128:(tt + 1) * 128,
                                  n2 * 512:(n2 + 1) * 512], st[:],
                        tail=True)

    _split_sync_waits(nc)
    return nc


_NC = None


def _get_nc():
    global _NC
    if _NC is None:
        _NC = _build_nc()
    return _NC


# ---------------------------------------------------------------------------
# Host entry point
# ---------------------------------------------------------------------------

def kernel(x, qkv_w, qkv_b, out_w, out_b):
    from concourse.bass_utils import run_bass_kernel_spmd

    trace_dir = os.environ.get("BASS_KERNEL_TRACE_DIR")
    if trace_dir:
        _install_ntff_shim()

    nc = _get_nc()

    x = np.asarray(x, np.float32)
    qkv_w = np.asarray(qkv_w, np.float32)
    qkv_b = np.asarray(qkv_b, np.float32)
    out_w = np.asarray(out_w, np.float32)
    out_b = np.asarray(out_b, np.float32)

    tri_np = np.broadcast_to(np.triu(np.ones((128, 128), np.float16))[:, None, :],
        (128, 2, 128)).copy()
    in_maps = []
    for c in range(NCORES):
        b, g = divmod(c, 4)
        fs = slice(256 * g, 256 * g + 256)
        wqkvT = np.ascontiguousarray(
            np.concatenate([qkv_w[0 * E:1 * E][fs],
                            qkv_w[1 * E:2 * E][fs],
                            qkv_w[2 * E:3 * E][fs]], axis=0).T)
        bq = np.concatenate([qkv_b[0 * E:1 * E][fs],
                             qkv_b[1 * E:2 * E][fs],
                             qkv_b[2 * E:3 * E][fs]])[:, None]
        in_maps.append({
            "xT": np.ascontiguousarray(x[b].T).astype(np.float16),
            "wqkvT": wqkvT.astype(np.float16),
            "woT": np.ascontiguousarray(out_w[:, fs].T).astype(np.float16),
            "bqkv": np.ascontiguousarray(bq),
            "tri": tri_np,
        })

    kwargs = {}
    if trace_dir:
        kwargs = {"trace": True, "tmpdir": trace_dir}
    res = run_bass_kernel_spmd(nc, in_maps, core_ids=list(range(NCORES)), **kwargs)
    if trace_dir and res.exec_time_ns is not None:
        print(f"HW exec time: {res.exec_time_ns} ns")

    out = np.zeros((B, N, E), np.float32)
    for c in range(NCORES):
        out[c // 4] += res.results[c]["outp"].astype(np.float32)
    out += out_b[None, None, :]
    return out

